# revision 38
# baseline (speedup 1.0000x reference)
"""Trainium2 Bass kernel for nn_CGPBlock (attention block with 1x1-conv QKV).

Reference computation (per batch b):
    q = Wq @ pose + bq; k = Wk @ id + bk; v = Wv @ pose + bv     # [C, L]
    energy[i, j] = sum_c q[c, i] k[c, j]                          # [L, L]
    attn = softmax_j(energy)
    va[c, i] = sum_j v[c, j] attn[i, j]
    out = pose + gamma * va

Sharding: data-parallel over batch, B=8 batches -> 8 NeuronCores (SPMD, no
collectives). Per core: C=128 fits the partition dim exactly, L=4096.

Device algorithm (per core, matmuls bf16 with fp32 PSUM accumulate):
  - q = WqT.T @ pose_bf + bq; k = WkT.T @ id_bf + bk (bf16, bias added on
    the PSUM->SBUF drain); v = WvT.T @ pose_bf with its bias folded into
    the output residual (attention rows sum to 1 after normalization).
  - vT j-tiles via blockwise DMA xbar transposes (off the PE critical path).
  - For each i-chunk (1024 cols), accumulate over 32 j-tiles:
      eT[j, i] = k_jt.T @ q_chunk         (PSUM, 2 x N=512 matmuls)
      pT = exp(eT)                        (one ACT op per j-tile; no max-sub:
                                           |E| < 32 so fp32 exp is safe, and
                                           softmax is shift-invariant)
      va[c, i] += vt_jt.T @ pT            (PSUM accumulate)
      Z[1, i]  += ones.T @ (pT quadsum)   (DVE pre-sums 4 j-tiles per M=1
                                           matmul: softmax column sums in the
                                           [j,i] layout, no transposes)
    then out = pose' + gamma * va * (1/Z), pose' = pose + gamma*bv.
  - Z is broadcast across partitions via a tiny DRAM round-trip (PE matmul
    broadcast on the last chunk, where there is no compute to hide the DMA
    latency under).

Scheduling notes (Tile executes each engine's stream in program order):
  - va/Z matmuls are emitted SKEW j-tiles behind the energy matmuls so the
    PE never waits on the exp.
  - Every chunked tensor is a separate tile per chunk: Tile tracks
    dependencies at tile granularity, so a single [C, L] tile would make
    chunk-0 consumers wait for all four chunk DMAs.
  - DMA queues: a consumer of a sync (HWDGE) DMA is released in queue-prefix
    order; gpsimd (SWDGE) waits release only when the whole queue drains.
    Deadline-critical loads go on sync in consumption order; pose (needed
    late) rides gpsimd.
  - ~3.5us of garbage matmuls pre-warm the PE clock gate (HAM) during the
    input DMAs so the convs run at 2.4 GHz instead of 1.2.
"""

import numpy as np
import ml_dtypes

import concourse.bacc as bacc
import concourse.tile as tile
from concourse import mybir
from concourse.bass_utils import run_bass_kernel_spmd

F32 = mybir.dt.float32
BF16 = mybir.dt.bfloat16
AF = mybir.ActivationFunctionType
ALU = mybir.AluOpType

B, C, L = 8, 128, 4096
CHUNK = 1024                # i-chunk width
NCH = L // CHUNK            # 4 chunks
NJT = L // 128              # 32 j-tiles
JPC = CHUNK // 128          # j-tiles per chunk tile
QUAD = 4                    # j-tiles pre-summed per Z matmul
SKEW = 8                    # software pipeline depth (PE runs ahead of ACT)

_CACHE = {}


def _build():
    nc = bacc.Bacc("TRN2", target_bir_lowering=False, debug=False, num_devices=B)

    pose_d = nc.dram_tensor("pose", [C, L], F32, kind="ExternalInput").ap()
    posebf_d = nc.dram_tensor("posebf", [C, L], BF16, kind="ExternalInput").ap()
    idbf_d = nc.dram_tensor("idbf", [C, L], BF16, kind="ExternalInput").ap()
    wt_d = nc.dram_tensor("wt", [C, 3 * C], BF16, kind="ExternalInput").ap()
    bq_d = nc.dram_tensor("bq", [C, 1], F32, kind="ExternalInput").ap()
    bk_d = nc.dram_tensor("bk", [C, 1], F32, kind="ExternalInput").ap()
    bfin_d = nc.dram_tensor("bfin", [C, 1], F32, kind="ExternalInput").ap()
    gam_d = nc.dram_tensor("gam", [C, 1], F32, kind="ExternalInput").ap()
    out_d = nc.dram_tensor("out", [C, L], F32, kind="ExternalOutput").ap()

    with tile.TileContext(nc) as tc:
        with tc.tile_pool(name="res", bufs=1) as res:
            wt_sb = res.tile([C, 3 * C], BF16)
            nc.sync.dma_start(wt_sb, wt_d)
            bq_sb = res.tile([C, 1], F32)
            bk_sb = res.tile([C, 1], F32)
            bfin_sb = res.tile([C, 1], F32)
            nc.gpsimd.dma_start(bfin_sb, bfin_d)
            gam_sb = res.tile([C, 1], F32)
            nc.gpsimd.dma_start(gam_sb, gam_d)
            ones_sb = res.tile([C, 1], BF16)
            nc.vector.memset(ones_sb, 1.0)
            onesr_sb = res.tile([1, C], F32)
            nc.vector.memset(onesr_sb, 1.0)

            def chunk_tiles(prefix, dtype):
                return [res.tile([C, CHUNK], dtype, name=f"{prefix}{i}")
                        for i in range(NCH)]

            pose_t = chunk_tiles("pose", F32)
            q_t = chunk_tiles("q", BF16)
            k_t = chunk_tiles("k", BF16)
            v_t = chunk_tiles("v", BF16)
            vt_t = chunk_tiles("vt", BF16)   # [j (partition), jt*128 + c]

            # Single-tile conv inputs: the conv phase releases at the input
            # batch's forwarding EVSEM anyway, and one big DMA costs one
            # descriptor-processing slot (~0.65us) instead of four.
            posebf_sb = res.tile([C, L], BF16)
            nc.sync.dma_start(posebf_sb, posebf_d)
            idbf_sb = res.tile([C, L], BF16)
            nc.sync.dma_start(idbf_sb, idbf_d)
            nc.sync.dma_start(bq_sb, bq_d)
            nc.sync.dma_start(bk_sb, bk_d)
            for ch in range(NCH):
                sl = slice(ch * CHUNK, (ch + 1) * CHUNK)
                nc.gpsimd.dma_start(pose_t[ch], pose_d[:, sl])

            # PE clock-gate pre-warm (no DMA deps): keep the PE busy with
            # garbage matmuls until the input DMAs land so the convs run at
            # 2.4 GHz (HAM 8/8) with no cold restart.
            warm_sb = res.tile([C, 512], BF16)
            nc.vector.memset(warm_sb, 0.0)
            with tc.tile_pool(name="warm_ps", bufs=1, space="PSUM") as warm_ps:
                wp = warm_ps.tile([1, 512], F32)
                for _ in range(14):
                    nc.tensor.matmul(wp, lhsT=ones_sb, rhs=warm_sb,
                                     start=True, stop=True)

            wqT = wt_sb[:, 0:C]
            wkT = wt_sb[:, C:2 * C]
            wvT = wt_sb[:, 2 * C:3 * C]

            # ---- QKV convs (1x1 = channel-mixing matmuls) ----
            # PSUM->SBUF drains: v/k on ACT (idle until the first exp),
            # q on DVE. bufs=3 leaves 2 virgin PSUM banks for attention.
            with tc.tile_pool(name="conv_ps", bufs=3, space="PSUM") as conv_ps:
                for ch in range(NCH):
                    vp = conv_ps.tile([C, CHUNK], F32, tag="cv", name="vp")
                    kp = conv_ps.tile([C, CHUNK], F32, tag="cv", name="kp")
                    qp = conv_ps.tile([C, CHUNK], F32, tag="cv", name="qp")
                    for h in range(CHUNK // 512):
                        hs = slice(h * 512, (h + 1) * 512)
                        sl = slice(ch * CHUNK + h * 512,
                                   ch * CHUNK + (h + 1) * 512)
                        nc.tensor.matmul(vp[:, hs], lhsT=wvT,
                                         rhs=posebf_sb[:, sl],
                                         start=True, stop=True)
                        nc.tensor.matmul(kp[:, hs], lhsT=wkT,
                                         rhs=idbf_sb[:, sl],
                                         start=True, stop=True)
                        nc.tensor.matmul(qp[:, hs], lhsT=wqT,
                                         rhs=posebf_sb[:, sl],
                                         start=True, stop=True)
                    nc.scalar.copy(v_t[ch], vp)
                    nc.scalar.activation(k_t[ch], kp, AF.Identity, bias=bk_sb)
                    nc.vector.tensor_scalar_add(q_t[ch], qp, bq_sb)
                    # vT tiles: one blockwise DMA xbar transpose per chunk
                    nc.sync.dma_start_transpose(
                        vt_t[ch].rearrange("p (t c) -> p t c", c=C),
                        v_t[ch])

            # pose' = pose + gamma*bv (per-partition const) — the residual
            for ch in range(NCH):
                nc.vector.tensor_scalar_add(pose_t[ch], pose_t[ch], bfin_sb)

            # ---- attention ----
            with (
                tc.tile_pool(name="et_ps", bufs=2, space="PSUM") as et_ps,
                tc.tile_pool(name="va_ps", bufs=1, space="PSUM") as va_ps,
                tc.tile_pool(name="z_ps", bufs=1, space="PSUM") as z_ps,
                tc.tile_pool(name="pt_sb", bufs=SKEW + QUAD + 2) as pt_pool,
                tc.tile_pool(name="qs_sb", bufs=2) as qs_pool,
                tc.tile_pool(name="nrm", bufs=2) as nrm,
                tc.tile_pool(name="outb", bufs=2) as outb,
                tc.tile_pool(name="dramp", bufs=2, space="DRAM") as dramp,
            ):
                for ch in range(NCH):
                    i0 = ch * CHUNK
                    isl = slice(i0, i0 + CHUNK)
                    va = va_ps.tile([C, CHUNK], F32)
                    z = z_ps.tile([1, CHUNK], F32)
                    pts = {}
                    # per-jt pipeline skew: SKEW steady, but decay to 2 near
                    # the end of the LAST chunk so the PE tail drains right
                    # behind the final exps instead of SKEW j-tiles later
                    def skew_at(j):
                        if ch < NCH - 1:
                            return SKEW
                        return max(2, min(SKEW, NJT + 2 - j))
                    lag_ptr = 0
                    for jt in range(NJT + 2 if ch == NCH - 1 else NJT + SKEW):
                        if jt < NJT:
                            ksl = slice((jt % JPC) * 128, (jt % JPC + 1) * 128)
                            et = et_ps.tile([C, CHUNK], F32)
                            for h in range(CHUNK // 512):
                                hs = slice(h * 512, (h + 1) * 512)
                                nc.tensor.matmul(
                                    et[:, hs], lhsT=k_t[jt // JPC][:, ksl],
                                    rhs=q_t[ch][:, hs],
                                    start=True, stop=True)
                            pt = pt_pool.tile([C, CHUNK], BF16)
                            nc.scalar.activation(pt, et, AF.Exp)
                            pts[jt] = pt
                        while lag_ptr <= min(jt - skew_at(jt), NJT - 1):
                            lag = lag_ptr
                            lag_ptr += 1
                            vsl = slice((lag % JPC) * 128, (lag % JPC + 1) * 128)
                            pt = pts[lag]
                            for h in range(CHUNK // 512):
                                hs = slice(h * 512, (h + 1) * 512)
                                nc.tensor.matmul(
                                    va[:, hs], lhsT=vt_t[lag // JPC][:, vsl],
                                    rhs=pt[:, hs],
                                    start=(lag == 0),
                                    stop=(lag == NJT - 1))
                            # Z column sums. In the kernel tail (last quad of
                            # the last chunk) use direct M=1 matmuls so Z
                            # doesn't wait on a serial DVE chain; elsewhere
                            # DVE pre-sums 4 pt tiles per Z matmul.
                            direct_z = (ch == NCH - 1
                                        and lag >= NJT - QUAD)
                            if direct_z:
                                pts.pop(lag)
                                for h in range(CHUNK // 512):
                                    hs = slice(h * 512, (h + 1) * 512)
                                    nc.tensor.matmul(z[0:1, hs], lhsT=ones_sb,
                                                     rhs=pt[:, hs],
                                                     start=False,
                                                     stop=(lag == NJT - 1))
                            elif lag % QUAD == QUAD - 1:
                                qd = lag // QUAD
                                z_stop = (ch < NCH - 1
                                          and qd == NJT // QUAD - 1)
                                p0, p1, p2, p3 = (pts.pop(lag - 3), pts.pop(lag - 2),
                                                  pts.pop(lag - 1), pts.pop(lag))
                                sa = qs_pool.tile([C, CHUNK], BF16, tag="sa")
                                nc.vector.tensor_add(sa, p0, p1)
                                sb_ = qs_pool.tile([C, CHUNK], BF16, tag="sb")
                                nc.vector.tensor_add(sb_, p2, p3)
                                sab = qs_pool.tile([C, CHUNK], BF16, tag="sab")
                                nc.vector.tensor_add(sab, sa, sb_)
                                for h in range(CHUNK // 512):
                                    hs = slice(h * 512, (h + 1) * 512)
                                    nc.tensor.matmul(z[0:1, hs], lhsT=ones_sb,
                                                     rhs=sab[:, hs],
                                                     start=(qd == 0),
                                                     stop=z_stop)

                    # free the va PSUM bank quickly, then normalize from SBUF
                    va_sb = nrm.tile([C, CHUNK], F32, tag="va_sb")
                    nc.vector.tensor_copy(va_sb, va)
                    if ch < NCH - 1:
                        rz = nrm.tile([1, CHUNK], F32, tag="rz")
                        nc.vector.reciprocal_approx_fast(rz, z)
                        # broadcast 1/Z across partitions via DRAM round-trip
                        # (latency hidden under the next chunk's compute)
                        zd = dramp.tile([1, CHUNK], F32)
                        nc.sync.dma_start(zd, rz)
                        rzb = nrm.tile([C, CHUNK], F32, tag="rzb")
                        nc.sync.dma_start(rzb, zd.to_broadcast([C, CHUNK]))
                        t = nrm.tile([C, CHUNK], F32, tag="t")
                        nc.vector.tensor_mul(t, va_sb, rzb)
                        o = outb.tile([C, CHUNK], F32)
                        nc.vector.scalar_tensor_tensor(
                            o, in0=t, scalar=gam_sb, in1=pose_t[ch],
                            op0=ALU.mult, op1=ALU.add)
                        nc.sync.dma_start(out_d[:, isl], o)
                    else:
                        # last chunk: nothing left to hide latency under —
                        # broadcast 1/Z on the (now idle) PE and drain in
                        # 512-wide half-pipelined steps
                        rz = nrm.tile([1, CHUNK], F32, tag="rz")
                        rzb = et_ps.tile([C, CHUNK], F32, tag="et",
                                         name="rzb_ps")
                        t = nrm.tile([C, CHUNK], F32, tag="t")
                        o = outb.tile([C, CHUNK], F32)
                        for h in range(CHUNK // 512):
                            hs = slice(h * 512, (h + 1) * 512)
                            ihs = slice(i0 + h * 512, i0 + (h + 1) * 512)
                            nc.vector.reciprocal_approx_fast(
                                rz[0:1, hs], z[0:1, hs])
                            nc.tensor.matmul(rzb[:, hs], lhsT=onesr_sb,
                                             rhs=rz[0:1, hs],
                                             start=True, stop=True)
                            nc.vector.tensor_mul(t[:, hs], va_sb[:, hs],
                                                 rzb[:, hs])
                            nc.vector.scalar_tensor_tensor(
                                o[:, hs], in0=t[:, hs], scalar=gam_sb,
                                in1=pose_t[ch][:, hs],
                                op0=ALU.mult, op1=ALU.add)
                            nc.sync.dma_start(out_d[:, ihs], o[:, hs])

    nc.compile()
    return nc


def _get_nc():
    if "nc" not in _CACHE:
        _CACHE["nc"] = _build()
    return _CACHE["nc"]


def kernel(pose_f, id_f, Wq, bq, Wk, bk, Wv, bv, gamma, **run_kwargs):
    pose_f = np.asarray(pose_f, dtype=np.float32)
    id_f = np.asarray(id_f, dtype=np.float32)
    Wq = np.asarray(Wq, dtype=np.float32)
    Wk = np.asarray(Wk, dtype=np.float32)
    Wv = np.asarray(Wv, dtype=np.float32)
    bq = np.asarray(bq, dtype=np.float32)
    bk = np.asarray(bk, dtype=np.float32)
    bv = np.asarray(bv, dtype=np.float32)
    g = float(np.asarray(gamma, dtype=np.float32).reshape(-1)[0])

    bf = ml_dtypes.bfloat16
    wt = np.concatenate([Wq.T, Wk.T, Wv.T], axis=1).astype(bf)  # [C_in, 3C]
    posebf = pose_f.astype(bf)
    idbf = id_f.astype(bf)
    bq_c = np.ascontiguousarray(bq.reshape(C, 1))
    bk_c = np.ascontiguousarray(bk.reshape(C, 1))
    bfin = np.ascontiguousarray((g * bv).reshape(C, 1).astype(np.float32))
    gam = np.full((C, 1), g, dtype=np.float32)

    in_maps = []
    for b in range(B):
        in_maps.append({
            "pose": pose_f[b],
            "posebf": posebf[b],
            "idbf": idbf[b],
            "wt": wt,
            "bq": bq_c,
            "bk": bk_c,
            "bfin": bfin,
            "gam": gam,
        })

    nc = _get_nc()
    res = run_bass_kernel_spmd(nc, in_maps, core_ids=list(range(B)), **run_kwargs)
    out = np.stack([res.results[b]["out"] for b in range(B)], axis=0)
    if run_kwargs:
        _CACHE["last_result"] = res
    return out


# revision 40
# speedup vs baseline: 1.0350x; 1.0350x over previous
"""Trainium2 Bass kernel for nn_CGPBlock (attention block with 1x1-conv QKV).

Reference computation (per batch b):
    q = Wq @ pose + bq; k = Wk @ id + bk; v = Wv @ pose + bv     # [C, L]
    energy[i, j] = sum_c q[c, i] k[c, j]                          # [L, L]
    attn = softmax_j(energy)
    va[c, i] = sum_j v[c, j] attn[i, j]
    out = pose + gamma * va

Sharding: data-parallel over batch, B=8 batches -> 8 NeuronCores (SPMD, no
collectives). Per core: C=128 fits the partition dim exactly, L=4096.

Device algorithm (per core, matmuls bf16 with fp32 PSUM accumulate):
  - q = WqT.T @ pose_bf + bq; k = WkT.T @ id_bf + bk (bf16, bias added on
    the PSUM->SBUF drain); v = WvT.T @ pose_bf with its bias folded into
    the output residual (attention rows sum to 1 after normalization).
  - vT j-tiles via blockwise DMA xbar transposes (off the PE critical path).
  - For each i-chunk (1024 cols), accumulate over 32 j-tiles:
      eT[j, i] = k_jt.T @ q_chunk         (PSUM, 2 x N=512 matmuls)
      pT = exp(eT)                        (one ACT op per j-tile; no max-sub:
                                           |E| < 32 so fp32 exp is safe, and
                                           softmax is shift-invariant)
      va[c, i] += vt_jt.T @ pT            (PSUM accumulate)
      Z[1, i]  += ones.T @ (pT quadsum)   (DVE pre-sums 4 j-tiles per M=1
                                           matmul: softmax column sums in the
                                           [j,i] layout, no transposes)
    then out = pose' + gamma * va * (1/Z), pose' = pose + gamma*bv.
  - Z is broadcast across partitions via a tiny DRAM round-trip (PE matmul
    broadcast on the last chunk, where there is no compute to hide the DMA
    latency under).

Scheduling notes (Tile executes each engine's stream in program order):
  - va/Z matmuls are emitted SKEW j-tiles behind the energy matmuls so the
    PE never waits on the exp.
  - Every chunked tensor is a separate tile per chunk: Tile tracks
    dependencies at tile granularity, so a single [C, L] tile would make
    chunk-0 consumers wait for all four chunk DMAs.
  - DMA queues: a consumer of a sync (HWDGE) DMA is released in queue-prefix
    order; gpsimd (SWDGE) waits release only when the whole queue drains.
    Deadline-critical loads go on sync in consumption order; pose (needed
    late) rides gpsimd.
  - ~3.5us of garbage matmuls pre-warm the PE clock gate (HAM) during the
    input DMAs so the convs run at 2.4 GHz instead of 1.2.
"""

import numpy as np
import ml_dtypes

import concourse.bacc as bacc
import concourse.tile as tile
from concourse import mybir
from concourse.bass_utils import run_bass_kernel_spmd

F32 = mybir.dt.float32
BF16 = mybir.dt.bfloat16
AF = mybir.ActivationFunctionType
ALU = mybir.AluOpType

B, C, L = 8, 128, 4096
CHUNK = 1024                # i-chunk width
NCH = L // CHUNK            # 4 chunks
NJT = L // 128              # 32 j-tiles
JPC = CHUNK // 128          # j-tiles per chunk tile
QUAD = 4                    # j-tiles pre-summed per Z matmul
SKEW = 8                    # software pipeline depth (PE runs ahead of ACT)

_CACHE = {}


def _build():
    nc = bacc.Bacc("TRN2", target_bir_lowering=False, debug=False, num_devices=B)

    pose_d = nc.dram_tensor("pose", [C, L], F32, kind="ExternalInput").ap()
    posebf_d = nc.dram_tensor("posebf", [C, L], BF16, kind="ExternalInput").ap()
    idbf_d = nc.dram_tensor("idbf", [C, L], BF16, kind="ExternalInput").ap()
    wt_d = nc.dram_tensor("wt", [C, 3 * C], BF16, kind="ExternalInput").ap()
    bq_d = nc.dram_tensor("bq", [C, 1], F32, kind="ExternalInput").ap()
    bk_d = nc.dram_tensor("bk", [C, 1], F32, kind="ExternalInput").ap()
    bfin_d = nc.dram_tensor("bfin", [C, 1], F32, kind="ExternalInput").ap()
    gam_d = nc.dram_tensor("gam", [C, 1], F32, kind="ExternalInput").ap()
    out_d = nc.dram_tensor("out", [C, L], F32, kind="ExternalOutput").ap()

    with tile.TileContext(nc) as tc:
        with tc.tile_pool(name="res", bufs=1) as res:
            wt_sb = res.tile([C, 3 * C], BF16)
            nc.sync.dma_start(wt_sb, wt_d)
            bq_sb = res.tile([C, 1], F32)
            bk_sb = res.tile([C, 1], F32)
            bfin_sb = res.tile([C, 1], F32)
            nc.gpsimd.dma_start(bfin_sb, bfin_d)
            gam_sb = res.tile([C, 1], F32)
            nc.gpsimd.dma_start(gam_sb, gam_d)
            ones_sb = res.tile([C, 1], BF16)
            nc.vector.memset(ones_sb, 1.0)
            onesr_sb = res.tile([1, C], F32)
            nc.vector.memset(onesr_sb, 1.0)

            def chunk_tiles(prefix, dtype):
                return [res.tile([C, CHUNK], dtype, name=f"{prefix}{i}")
                        for i in range(NCH)]

            pose_t = chunk_tiles("pose", F32)
            posebf_t = chunk_tiles("posebf", BF16)
            idbf_t = chunk_tiles("idbf", BF16)
            q_t = chunk_tiles("q", BF16)
            k_t = chunk_tiles("k", BF16)
            v_t = chunk_tiles("v", BF16)
            vt_t = chunk_tiles("vt", BF16)   # [j (partition), jt*128 + c]

            for ch in range(NCH):
                sl = slice(ch * CHUNK, (ch + 1) * CHUNK)
                nc.sync.dma_start(posebf_t[ch], posebf_d[:, sl])
                nc.sync.dma_start(idbf_t[ch], idbf_d[:, sl])
                if ch == 1:
                    # biases slot in after the chunk-1 loads (their consumers,
                    # the PSUM bias-drains, run later than the chunk-1 convs)
                    nc.sync.dma_start(bq_sb, bq_d)
                    nc.sync.dma_start(bk_sb, bk_d)
            for ch in range(NCH):
                sl = slice(ch * CHUNK, (ch + 1) * CHUNK)
                nc.gpsimd.dma_start(pose_t[ch], pose_d[:, sl])

            # PE clock-gate pre-warm (no DMA deps): keep the PE busy with
            # garbage matmuls until the input DMAs land so the convs run at
            # 2.4 GHz (HAM 8/8) with no cold restart.
            warm_sb = res.tile([C, 512], BF16)
            nc.vector.memset(warm_sb, 0.0)
            with tc.tile_pool(name="warm_ps", bufs=1, space="PSUM") as warm_ps:
                wp = warm_ps.tile([1, 512], F32)
                for _ in range(20):
                    nc.tensor.matmul(wp, lhsT=ones_sb, rhs=warm_sb,
                                     start=True, stop=True)

            wqT = wt_sb[:, 0:C]
            wkT = wt_sb[:, C:2 * C]
            wvT = wt_sb[:, 2 * C:3 * C]

            # ---- QKV convs (1x1 = channel-mixing matmuls) ----
            # PSUM->SBUF drains: v/k on ACT (idle until the first exp),
            # q on DVE. bufs=3 leaves 2 virgin PSUM banks for attention.
            with tc.tile_pool(name="conv_ps", bufs=3, space="PSUM") as conv_ps:
                for ch in range(NCH):
                    vp = conv_ps.tile([C, CHUNK], F32, tag="cv", name="vp")
                    kp = conv_ps.tile([C, CHUNK], F32, tag="cv", name="kp")
                    qp = conv_ps.tile([C, CHUNK], F32, tag="cv", name="qp")
                    for h in range(CHUNK // 512):
                        hs = slice(h * 512, (h + 1) * 512)
                        nc.tensor.matmul(vp[:, hs], lhsT=wvT,
                                         rhs=posebf_t[ch][:, hs],
                                         start=True, stop=True)
                        nc.tensor.matmul(kp[:, hs], lhsT=wkT,
                                         rhs=idbf_t[ch][:, hs],
                                         start=True, stop=True)
                        nc.tensor.matmul(qp[:, hs], lhsT=wqT,
                                         rhs=posebf_t[ch][:, hs],
                                         start=True, stop=True)
                    nc.scalar.copy(v_t[ch], vp)
                    nc.scalar.activation(k_t[ch], kp, AF.Identity, bias=bk_sb)
                    nc.vector.tensor_scalar_add(q_t[ch], qp, bq_sb)
                    # vT tiles: one blockwise DMA xbar transpose per chunk
                    nc.sync.dma_start_transpose(
                        vt_t[ch].rearrange("p (t c) -> p t c", c=C),
                        v_t[ch])

            # pose' = pose + gamma*bv (per-partition const) — the residual
            for ch in range(NCH):
                nc.vector.tensor_scalar_add(pose_t[ch], pose_t[ch], bfin_sb)

            # ---- attention ----
            with (
                tc.tile_pool(name="et_ps", bufs=2, space="PSUM") as et_ps,
                tc.tile_pool(name="va_ps", bufs=1, space="PSUM") as va_ps,
                tc.tile_pool(name="z_ps", bufs=1, space="PSUM") as z_ps,
                tc.tile_pool(name="pt_sb", bufs=SKEW + QUAD + 2) as pt_pool,
                tc.tile_pool(name="qs_sb", bufs=2) as qs_pool,
                tc.tile_pool(name="nrm", bufs=2) as nrm,
                tc.tile_pool(name="outb", bufs=2) as outb,
                tc.tile_pool(name="dramp", bufs=2, space="DRAM") as dramp,
            ):
                for ch in range(NCH):
                    i0 = ch * CHUNK
                    isl = slice(i0, i0 + CHUNK)
                    va = va_ps.tile([C, CHUNK], F32)
                    z = z_ps.tile([1, CHUNK], F32)
                    pts = {}
                    # per-jt pipeline skew: SKEW steady, but decay to 2 near
                    # the end of the LAST chunk so the PE tail drains right
                    # behind the final exps instead of SKEW j-tiles later
                    def skew_at(j):
                        if ch < NCH - 1:
                            return SKEW
                        return max(2, min(SKEW, NJT + 2 - j))
                    lag_ptr = 0
                    for jt in range(NJT + 2 if ch == NCH - 1 else NJT + SKEW):
                        if jt < NJT:
                            ksl = slice((jt % JPC) * 128, (jt % JPC + 1) * 128)
                            et = et_ps.tile([C, CHUNK], F32)
                            for h in range(CHUNK // 512):
                                hs = slice(h * 512, (h + 1) * 512)
                                nc.tensor.matmul(
                                    et[:, hs], lhsT=k_t[jt // JPC][:, ksl],
                                    rhs=q_t[ch][:, hs],
                                    start=True, stop=True)
                            pt = pt_pool.tile([C, CHUNK], BF16)
                            nc.scalar.activation(pt, et, AF.Exp)
                            pts[jt] = pt
                        while lag_ptr <= min(jt - skew_at(jt), NJT - 1):
                            lag = lag_ptr
                            lag_ptr += 1
                            vsl = slice((lag % JPC) * 128, (lag % JPC + 1) * 128)
                            pt = pts[lag]
                            for h in range(CHUNK // 512):
                                hs = slice(h * 512, (h + 1) * 512)
                                nc.tensor.matmul(
                                    va[:, hs], lhsT=vt_t[lag // JPC][:, vsl],
                                    rhs=pt[:, hs],
                                    start=(lag == 0),
                                    stop=(lag == NJT - 1))
                            # Z column sums. In the kernel tail (last quad of
                            # the last chunk) use direct M=1 matmuls so Z
                            # doesn't wait on a serial DVE chain; elsewhere
                            # DVE pre-sums 4 pt tiles per Z matmul.
                            direct_z = (ch == NCH - 1
                                        and lag >= NJT - QUAD)
                            if direct_z:
                                pts.pop(lag)
                                for h in range(CHUNK // 512):
                                    hs = slice(h * 512, (h + 1) * 512)
                                    nc.tensor.matmul(z[0:1, hs], lhsT=ones_sb,
                                                     rhs=pt[:, hs],
                                                     start=False,
                                                     stop=(lag == NJT - 1))
                            elif lag % QUAD == QUAD - 1:
                                qd = lag // QUAD
                                p0, p1, p2, p3 = (pts.pop(lag - 3), pts.pop(lag - 2),
                                                  pts.pop(lag - 1), pts.pop(lag))
                                sa = qs_pool.tile([C, CHUNK], BF16, tag="sa")
                                nc.vector.tensor_add(sa, p0, p1)
                                sb_ = qs_pool.tile([C, CHUNK], BF16, tag="sb")
                                nc.vector.tensor_add(sb_, p2, p3)
                                sab = qs_pool.tile([C, CHUNK], BF16, tag="sab")
                                nc.vector.tensor_add(sab, sa, sb_)
                                # pair adjacent quad-sums (one more DVE add)
                                # so each M=1 Z matmul covers 8 j-tiles:
                                # halves the Z matmul count, keeping PE/jt
                                # under the ACT exp pace
                                last_q = (NJT // QUAD - 1) if ch < NCH - 1 \
                                    else (NJT - QUAD) // QUAD - 1
                                if qd % 2 == 0 and qd != last_q:
                                    sab_prev = sab
                                    src = None
                                elif qd % 2 == 1:
                                    s8 = qs_pool.tile([C, CHUNK], BF16,
                                                      tag="s8")
                                    nc.vector.tensor_add(s8, sab_prev, sab)
                                    src = s8
                                else:
                                    src = sab   # odd quad-count tail flush
                                if src is not None:
                                    z_stop = (ch < NCH - 1
                                              and qd == NJT // QUAD - 1)
                                    for h in range(CHUNK // 512):
                                        hs = slice(h * 512, (h + 1) * 512)
                                        nc.tensor.matmul(z[0:1, hs],
                                                         lhsT=ones_sb,
                                                         rhs=src[:, hs],
                                                         start=(qd <= 1),
                                                         stop=z_stop)

                    # free the va PSUM bank quickly, then normalize from SBUF
                    va_sb = nrm.tile([C, CHUNK], F32, tag="va_sb")
                    nc.vector.tensor_copy(va_sb, va)
                    if ch < NCH - 1:
                        rz = nrm.tile([1, CHUNK], F32, tag="rz")
                        nc.vector.reciprocal_approx_fast(rz, z)
                        # broadcast 1/Z across partitions via DRAM round-trip
                        # (latency hidden under the next chunk's compute)
                        zd = dramp.tile([1, CHUNK], F32)
                        nc.sync.dma_start(zd, rz)
                        rzb = nrm.tile([C, CHUNK], F32, tag="rzb")
                        nc.sync.dma_start(rzb, zd.to_broadcast([C, CHUNK]))
                        t = nrm.tile([C, CHUNK], F32, tag="t")
                        nc.vector.tensor_mul(t, va_sb, rzb)
                        o = outb.tile([C, CHUNK], F32)
                        nc.vector.scalar_tensor_tensor(
                            o, in0=t, scalar=gam_sb, in1=pose_t[ch],
                            op0=ALU.mult, op1=ALU.add)
                        nc.sync.dma_start(out_d[:, isl], o)
                    else:
                        # last chunk: nothing left to hide latency under —
                        # broadcast 1/Z on the (now idle) PE and drain in
                        # 512-wide half-pipelined steps
                        rz = nrm.tile([1, CHUNK], F32, tag="rz")
                        rzb = et_ps.tile([C, CHUNK], F32, tag="et",
                                         name="rzb_ps")
                        t = nrm.tile([C, CHUNK], F32, tag="t")
                        o = outb.tile([C, CHUNK], F32)
                        for h in range(CHUNK // 512):
                            hs = slice(h * 512, (h + 1) * 512)
                            ihs = slice(i0 + h * 512, i0 + (h + 1) * 512)
                            nc.vector.reciprocal_approx_fast(
                                rz[0:1, hs], z[0:1, hs])
                            nc.tensor.matmul(rzb[:, hs], lhsT=onesr_sb,
                                             rhs=rz[0:1, hs],
                                             start=True, stop=True)
                            nc.vector.tensor_mul(t[:, hs], va_sb[:, hs],
                                                 rzb[:, hs])
                            nc.vector.scalar_tensor_tensor(
                                o[:, hs], in0=t[:, hs], scalar=gam_sb,
                                in1=pose_t[ch][:, hs],
                                op0=ALU.mult, op1=ALU.add)
                            nc.sync.dma_start(out_d[:, ihs], o[:, hs])

    nc.compile()
    return nc


def _get_nc():
    if "nc" not in _CACHE:
        _CACHE["nc"] = _build()
    return _CACHE["nc"]


def kernel(pose_f, id_f, Wq, bq, Wk, bk, Wv, bv, gamma, **run_kwargs):
    pose_f = np.asarray(pose_f, dtype=np.float32)
    id_f = np.asarray(id_f, dtype=np.float32)
    Wq = np.asarray(Wq, dtype=np.float32)
    Wk = np.asarray(Wk, dtype=np.float32)
    Wv = np.asarray(Wv, dtype=np.float32)
    bq = np.asarray(bq, dtype=np.float32)
    bk = np.asarray(bk, dtype=np.float32)
    bv = np.asarray(bv, dtype=np.float32)
    g = float(np.asarray(gamma, dtype=np.float32).reshape(-1)[0])

    bf = ml_dtypes.bfloat16
    wt = np.concatenate([Wq.T, Wk.T, Wv.T], axis=1).astype(bf)  # [C_in, 3C]
    posebf = pose_f.astype(bf)
    idbf = id_f.astype(bf)
    bq_c = np.ascontiguousarray(bq.reshape(C, 1))
    bk_c = np.ascontiguousarray(bk.reshape(C, 1))
    bfin = np.ascontiguousarray((g * bv).reshape(C, 1).astype(np.float32))
    gam = np.full((C, 1), g, dtype=np.float32)

    in_maps = []
    for b in range(B):
        in_maps.append({
            "pose": pose_f[b],
            "posebf": posebf[b],
            "idbf": idbf[b],
            "wt": wt,
            "bq": bq_c,
            "bk": bk_c,
            "bfin": bfin,
            "gam": gam,
        })

    nc = _get_nc()
    res = run_bass_kernel_spmd(nc, in_maps, core_ids=list(range(B)), **run_kwargs)
    out = np.stack([res.results[b]["out"] for b in range(B)], axis=0)
    if run_kwargs:
        _CACHE["last_result"] = res
    return out
